# revision 9
# baseline (speedup 1.0000x reference)
"""AttentionCritic Trainium2 kernel — 8-core SPMD, head/query-half sharded.

Math restructuring (exact up to fp assoc.):
  mask[i,j] = (|x_i-x_j|<=4)&(|y_i-y_j|<=2)&(j>i)          (host, from int state)
  C' = [act(128), obs(16), 1]  (ones row folds all biases)
  S_h = C' G' C'^T / 12 with G' = Aq Ak^T host-folded [145,145]
        (Aq = [Wq_eff rows | bq_eff] etc. — the two-stage reference
        projections (C@Wq+bq)@Wiq+biq are first folded to single eff mats)
  T1 = C' G' computed X-form: T1^T = G''^T [h;a]^T + bT1 (G'' absorbs the
        obs encoder for T1 only; bT1 = ones-row of G', added in the
        PSUM->SBUF copy as a per-partition bias)
  E_h = exp(S_h)  (softmax ratio is shift-invariant; |S| small enough that
        exp is safely fp32 — validated on this data by earlier versions)
  D[j,i] = sum_k E[j,k] mask[i,k];  R = mask^T/max(D,1e-9)  (approx recip)
  W[k,i] = mask[i,k] * sum_j E[j,k] R[j,i]
  u = C' @ (Av W_out_h)  [256,5]  (v/ctx/out-proj fold: Q_p = sum_k W[k,i] u[k,a])
  Q = sum_cores Q_p^T + n_i*c1 + c2  (host; W_out = Wo_eff @ Wdueling [576,5])

Sharding: core c handles (head h=c//2, query-half jm=c%2). Everything after
exp is linear in j and h, so each core emits a partial Q^T [5,256] over ALL
256 agents (free dim 256 keeps fp32r matmuls at the fast 1 cyc/row rate) and
the host sums the 8 partials. The j-half selection is uniform across cores:
the per-core input packing rotates the agent axis by 128*jm, so slice
[0:128] is always "my" j-half.

Per-core: 19 matmuls / ~3.9K stream rows, ~700KB DMA.
"""

import sys

for _p in ("/opt/trn_rl_repo",):
    if _p not in sys.path:
        sys.path.append(_p)

import contextlib

import numpy as np

import concourse.bass as bass
import concourse.bacc as bacc
import concourse.mybir as mybir
from concourse.tile import TileContext
from concourse import bass_utils

N, HID, ACT, NH = 256, 128, 5, 4
D, E, HD = 144, 576, 144
NCORES = 8
F32 = mybir.dt.float32
F32R = mybir.dt.float32r
SCALE = 1.0 / 12.0
CF = 145  # C' feature dim: act(128) + obs(16) + ones(1)

# blob1 [128, B1_COLS] column layout (host packing must match kernel slicing)
# dma1: G''A(145) hTp(256) G''B(145)   -> cols 0:546
# dma2: aTp(256) wenc(16)               -> cols 546:818
# dma3: mT0(256) mT1(256) WvaugA(8, padded) bT1m(1) id128(128) -> cols 818:1467
B1_GA, B1_HT, B1_GB = 0, 145, 401
B1_D1 = 546
B1_AT, B1_WENC = 546, 802
B1_D2 = 818
B1_MT0, B1_MT1, B1_WVA, B1_BT1 = 818, 1074, 1330, 1338
B1_ID = 1339
B1_COLS = 1467
# blob2 [17, 10]: WvaugB(8, padded from 5) benc(1, rows 0:16) bT1t(1)
B2_WVB, B2_BENC, B2_BT1T = 0, 8, 9
B2_COLS = 10


def _build():
    nc = bacc.Bacc(target_bir_lowering=False)

    def dp(name, shape, dtype, isOutput=False):
        return nc.declare_dram_parameter(name, shape, dtype, isOutput)

    b1_d = dp("blob1", [128, B1_COLS], F32)
    b2_d = dp("blob2", [17, B2_COLS], F32)
    out_d = dp("out", [5, N], F32, isOutput=True)

    with TileContext(nc) as tc:
        with contextlib.ExitStack() as ctx:
            wp = ctx.enter_context(tc.tile_pool(name="wp", bufs=1))
            pp = ctx.enter_context(tc.tile_pool(name="pp", bufs=8, space="PSUM"))

            def wt(shape, tag, dtype=F32R):
                return wp.tile(shape, dtype, tag=tag, name=tag)

            def ps(shape, dtype=F32):
                return pp.tile(shape, dtype, tag="mm", name="mm")

            # ---------- DMAs: two sync-queue chunks (need order), one on
            # scalar (later-needed), blob2 via SWDGE (independent path) ----
            b1 = wt([128, B1_COLS], "b1")
            nc.sync.dma_start(out=b1[:, 0:B1_D1],
                              in_=b1_d[:, 0:B1_D1].bitcast(F32R))
            nc.scalar.dma_start(out=b1[:, B1_D1:B1_D2],
                                in_=b1_d[:, B1_D1:B1_D2].bitcast(F32R))
            b2 = wt([17, B2_COLS], "b2")
            nc.gpsimd.dma_start(out=b2, in_=b2_d[:, :].bitcast(F32R))
            nc.gpsimd.dma_start(out=b1[:, B1_D2:B1_COLS],
                                in_=b1_d[:, B1_D2:B1_COLS].bitcast(F32R))

            GA = b1[:, B1_GA:B1_GA + CF]
            hTp = b1[:, B1_HT:B1_HT + N]
            wenc = b1[:, B1_WENC:B1_WENC + 16]
            GB = b1[:, B1_GB:B1_GB + CF]
            aTp = b1[:, B1_AT:B1_AT + N]
            mT = [b1[:, B1_MT0:B1_MT0 + N], b1[:, B1_MT1:B1_MT1 + N]]
            WvaugA = b1[:, B1_WVA:B1_WVA + 8]
            bT1m = b1[:, B1_BT1:B1_BT1 + 1].bitcast(F32)
            ident = b1[:, B1_ID:B1_ID + 128]
            WvaugB = b2[:, B2_WVB:B2_WVB + 8]
            benc = b2[0:16, B2_BENC:B2_BENC + 1].bitcast(F32)
            bT1t = b2[:, B2_BT1T:B2_BT1T + 1].bitcast(F32)

            # ---------- T1^T = G''^T [h;a]^T + bT1: [128,256] + [17,256] ----
            ctT_full = wt([32, N], "ctT")
            nc.vector.memset(ctT_full[:, :].bitcast(F32), 1.0)
            ctT = ctT_full[0:17, :]
            pT1m = ps([128, N])
            nc.tensor.matmul(pT1m, GA[:, 0:128], hTp, start=True, stop=False)
            nc.tensor.matmul(pT1m, GB[:, 0:128], aTp, start=False, stop=True)
            pT1t = ps([17, N])
            nc.tensor.matmul(pT1t, GA[:, 128:CF], hTp, start=True, stop=False)
            nc.tensor.matmul(pT1t, GB[:, 128:CF], aTp, start=False, stop=True)
            # obs row block of C' (off critical path; PE idles here anyway)
            pObs = ps([16, N])
            nc.tensor.matmul(pObs, wenc, hTp, start=True, stop=True)
            T1m = wt([128, N], "T1m")
            nc.vector.tensor_scalar(T1m, pT1m, bT1m, None, mybir.AluOpType.add)
            T1t = wt([17, N], "T1t")
            nc.vector.tensor_scalar(T1t, pT1t, bT1t, None, mybir.AluOpType.add)
            nc.vector.tensor_scalar(ctT_full[0:16, :], pObs, benc, None,
                                    mybir.AluOpType.add)

            # ---------- S = T1_jhalf C'^T -> E = exp(S/12) (2 halves) ----
            pD = ps([128, N])
            nc.vector.memset(pD, 1e-9)
            pS = ps([128, N])
            nc.tensor.matmul(pS, T1m[:, 0:128], aTp, start=True, stop=False)
            nc.tensor.matmul(pS, T1t[:, 0:128], ctT, start=False, stop=True)
            Et = wt([128, N], "Et")
            nc.scalar.activation(Et[:, 0:128], pS[:, 0:128],
                                 mybir.ActivationFunctionType.Exp, scale=SCALE)
            nc.scalar.activation(Et[:, 128:N], pS[:, 128:N],
                                 mybir.ActivationFunctionType.Exp, scale=SCALE)

            # ---------- E^T via PE transpose (exp(S)^T == exp(S^T)) ----------
            ET = []
            for kc in range(2):
                pT = ps([128, 128], F32R)
                nc.tensor.transpose(pT, Et[:, kc * 128:(kc + 1) * 128], ident)
                t = wt([128, 128], f"ET{kc}")
                nc.vector.tensor_copy(out=t, in_=pT)
                ET.append(t)

            # ---------- D (eps preloaded); R = mask^T * approx_recip(D) ----
            nc.tensor.matmul(pD, ET[0], mT[0], start=False, stop=False,
                             skip_group_check=True)
            nc.tensor.matmul(pD, ET[1], mT[1], start=False, stop=True,
                             skip_group_check=True)
            Rr = wt([128, N], "Rr", F32)
            nc.vector.reciprocal_approx_fast(out=Rr, in_=pD)
            R = wt([128, N], "R")
            nc.vector.tensor_tensor(R, Rr.bitcast(F32R), mT[0],
                                    mybir.AluOpType.mult)

            # ---------- u = C' @ Wv_aug : [128,5] x2 k-chunks ----------
            u_t = []
            for kc in range(2):
                pu = ps([128, 8])
                nc.tensor.matmul(pu, aTp[:, kc * 128:(kc + 1) * 128], WvaugA,
                                 start=True, stop=False)
                nc.tensor.matmul(pu, ctT[:, kc * 128:(kc + 1) * 128], WvaugB,
                                 start=False, stop=True)
                t = wt([128, 8], f"u{kc}")
                nc.vector.tensor_copy(out=t, in_=pu)
                u_t.append(t)

            # ---------- W = mask^T * (E^T-partial over my j-half) ----------
            Wt = []
            for kc in range(2):
                pW = ps([128, N])
                nc.tensor.matmul(pW, Et[:, kc * 128:(kc + 1) * 128], R,
                                 start=True, stop=True)
                t = wt([128, N], f"W{kc}")
                nc.vector.tensor_tensor(t, pW, mT[kc].bitcast(F32),
                                        mybir.AluOpType.mult)
                Wt.append(t)

            # ---------- partial Q^T = u^T-contract with W : [5,256] ----------
            pQ = ps([8, N])
            nc.tensor.matmul(pQ, u_t[0], Wt[0], start=True, stop=False)
            nc.tensor.matmul(pQ, u_t[1], Wt[1], start=False, stop=True)
            Qsb = wt([8, N], "Qsb", F32)
            nc.vector.tensor_copy(out=Qsb, in_=pQ)
            nc.sync.dma_start(out=out_d[:, :], in_=Qsb[0:5, :])

    nc.compile()
    return nc


_NC_CACHE = {}


def _make_in_maps(inputs):
    f32 = np.float32
    g = lambda k: np.asarray(inputs[k], dtype=np.float64)

    hidden = np.asarray(inputs["hidden_state_n"], dtype=f32)
    action = np.asarray(inputs["action_n"], dtype=f32)
    state = np.asarray(inputs["state_n"]).astype(np.int64)

    # host-side weight folding (float64, cast to f32 at the end)
    Wq_eff = g("Wq") @ g("Wiq")
    bq_eff = g("bq") @ g("Wiq") + g("biq")
    Wk_eff = g("Wk") @ g("Wik")
    bk_eff = g("bk") @ g("Wik") + g("bik")
    Wv_eff = g("Wv") @ g("Wiv")
    bv_eff = g("bv") @ g("Wiv") + g("biv")
    Wo_eff = g("Wo_proj") @ g("W_O")          # [576,144]
    bo_eff = g("bo_proj") @ g("W_O")          # [144]
    W_adv = g("W_adv")
    W_Q = (g("W_val") @ np.ones((1, ACT)) + W_adv
           - (W_adv @ np.ones((ACT, ACT))) / ACT)              # [144,5]
    b_Q = g("b_val")[0] + g("b_adv") - g("b_adv").mean()       # [5]
    W_out = Wo_eff @ W_Q                                       # [576,5]
    c1 = (bo_eff @ W_Q).astype(f32)                            # [5]
    c2 = b_Q.astype(f32)                                       # [5]

    # mask from int state (host): mask[i,j] = j observed by i
    dx = np.abs(state[:, None, 0] - state[None, :, 0])
    dy = np.abs(state[:, None, 1] - state[None, :, 1])
    upper = np.arange(N)[None, :] > np.arange(N)[:, None]
    mask = ((dx <= 4) & (dy <= 2) & upper).astype(f32)         # [N,N]
    n_i = mask.sum(axis=1)                                     # [N]
    maskT = np.ascontiguousarray(mask.T)                       # [j,i]

    W_enc = g("W_enc")                                         # [128,16]
    b_enc = np.asarray(inputs["b_enc"], dtype=f32)             # [16]
    hT = np.ascontiguousarray(hidden.T)                        # [128,256]
    aT = np.ascontiguousarray(action.T)

    in_maps = []
    for c in range(NCORES):
        h, jm = c // 2, c % 2
        perm = np.roll(np.arange(N), -jm * 128)
        cols = slice(144 * h, 144 * h + 144)
        # A-mats in C'-feature row order [act(128), obs(16), ones(1)]
        def amat(W, b):
            Wh, bh = W[:, cols], b[cols]
            return np.vstack([Wh[16:144], Wh[0:16], bh[None, :]])  # [145,144]
        Aq, Ak, Av = amat(Wq_eff, bq_eff), amat(Wk_eff, bk_eff), \
            amat(Wv_eff, bv_eff)
        Gp = Aq @ Ak.T                                         # [145,145]
        GppA = W_enc @ Gp[128:144, :]                          # hid rows [128,145]
        GppB = Gp[0:128, :]                                    # act rows [128,145]
        bT1 = Gp[144, :]                                       # [145]
        Wv_aug = np.concatenate([Av @ W_out[cols, :],
                                 np.zeros((CF, 3))], axis=1)   # [145,8] padded
        mTp = maskT[perm, :]
        b1 = np.concatenate([
            GppA.astype(f32), hT[:, perm], GppB.astype(f32),
            aT[:, perm], W_enc.astype(f32),
            mTp[0:128], mTp[128:256], Wv_aug[0:128].astype(f32),
            bT1[0:128].astype(f32).reshape(128, 1),
            np.eye(128, dtype=f32)], axis=1)
        b2 = np.concatenate([
            Wv_aug[128:145].astype(f32),
            np.concatenate([b_enc.reshape(16, 1), np.zeros((1, 1), f32)]),
            bT1[128:145].astype(f32).reshape(17, 1)], axis=1)
        in_maps.append({
            "blob1": np.ascontiguousarray(b1, dtype=f32),
            "blob2": np.ascontiguousarray(b2, dtype=f32),
        })
    return in_maps, n_i, c1, c2


def kernel(**inputs):
    if "nc" not in _NC_CACHE:
        _NC_CACHE["nc"] = _build()
    nc = _NC_CACHE["nc"]
    in_maps, n_i, c1, c2 = _make_in_maps(inputs)
    res = bass_utils.run_bass_kernel_spmd(nc, in_maps, core_ids=list(range(NCORES)))
    QT = np.zeros((ACT, N), np.float32)
    for c in range(NCORES):
        QT += res.results[c]["out"]
    Q = QT.T + n_i[:, None] * c1[None, :] + c2[None, :]
    return Q.astype(np.float32)


# revision 15
# speedup vs baseline: 1.1231x; 1.1231x over previous
"""AttentionCritic Trainium2 kernel — 8-core SPMD, head/query-half sharded,
bf16 compute with fp32 PSUM accumulation.

Math restructuring (exact up to fp assoc./bf16 rounding):
  mask[i,j] = (|x_i-x_j|<=4)&(|y_i-y_j|<=2)&(j>i)          (host, from int state)
  C' = [act(128), obs(16), 1]  (ones feature folds all biases)
  S_h = C' G' C'^T / 12,  G' = Aq Ak^T host-folded from the two-stage
        reference projections (C@Wq+bq)@Wiq+biq -> single eff mats + bias rows
  T1aug = C' [G' | Wv_aug]  computed X-form: T1aug^T = G''^T [h;a]^T + bias
        (G'' absorbs the obs encoder; bias row added in the PSUM->SBUF copy).
        Wv_aug = Av (Wo_eff Wduel)_head, so T1aug's tail also carries
        u^T = (C' Wv_aug)^T — the v/ctx/out-proj fold rides along for free.
  E_h = exp(S_h)  (softmax ratio is shift-invariant; |S/12|<~3 so bf16
        logits cost <~1% on exp)
  E^T via PE transpose;  u via PE transpose of T1aug tail rows
  D[j,i] = sum_k E[j,k] mask[i,k] + 1e-9 (eps preloaded in PSUM)
  R = mask^T * approx_recip(D);  W[k,i] = mask[i,k] * sum_j E[j,k] R[j,i]
  Q_p^T[a,i] = sum_k u[k,a] W[k,i]
  Q = sum_cores Q_p^T + n_i*c1 + c2  (host)

Sharding: core c handles (head h=c//2, query-half jm=c%2). Everything after
exp is linear in j and h, so each core emits a partial Q^T [5,256] over ALL
256 agents and the host sums the 8 partials. The j-half selection is uniform
across cores: the per-core input packing rotates the agent axis by 128*jm,
so slice [0:128] is always "my" j-half.

Per-core: 17 matmuls, ~390KB DMA, all matmuls bf16 (1 cyc/row — fp32r runs
4x slower as fp32_mode=HIGH on this part), accumulation in fp32 PSUM.
"""

import sys

for _p in ("/opt/trn_rl_repo",):
    if _p not in sys.path:
        sys.path.append(_p)

import contextlib

import numpy as np
import ml_dtypes

import concourse.bass as bass
import concourse.bacc as bacc
import concourse.mybir as mybir
from concourse.tile import TileContext
from concourse import bass_utils

N, HID, ACT, NH = 256, 128, 5, 4
D, E, HD = 144, 576, 144
NCORES = 8
F32 = mybir.dt.float32
BF16 = mybir.dt.bfloat16
BF16NP = ml_dtypes.bfloat16
SCALE = 1.0 / 12.0
CF = 145   # C' feature dim: act(128) + obs(16) + ones(1)
# T1aug^T tail tile layout: rows 0:8 = u^T (padded from 5), rows 8:32 zero
# pad (transpose/matmul partition starts must be 32-aligned), rows 32:49 =
# T1 tail features (obs 16 + ones 1)
TAUG = 177  # G'aug cols: main(128) + u(8) + pad(24) + T1tail(17)
TTL = 49    # tail tile partitions

# blob1 [128, B1_COLS] bf16 column layout (host packing must match)
# dma1 (sync):   GA(177) hTp(256) bT1m(1) btail(1)      -> 0:435
# dma2 (scalar): GB(177) aTp(256) wenc(16) benc(1)      -> 435:885
# dma3 (gpsimd): mT0(256) mT1(256) id128(128)           -> 885:1525
B1_GA, B1_HT, B1_BT1M, B1_BTL = 0, 177, 433, 434
B1_D1 = 435
B1_GB, B1_AT, B1_WENC, B1_BENC = 435, 612, 868, 884
B1_D2 = 885
B1_MT0, B1_MT1, B1_ID = 885, 1141, 1397
B1_COLS = 1525


def _build():
    nc = bacc.Bacc(target_bir_lowering=False)

    b1_d = nc.declare_dram_parameter("blob1", [128, B1_COLS], BF16, False)
    b2_d = nc.declare_dram_parameter("blob2", [128, 3], F32, False)
    out_d = nc.declare_dram_parameter("out", [ACT, N], F32, True)

    with TileContext(nc) as tc:
        with contextlib.ExitStack() as ctx:
            wp = ctx.enter_context(tc.tile_pool(name="wp", bufs=1))
            pp = ctx.enter_context(tc.tile_pool(name="pp", bufs=8, space="PSUM"))

            def wt(shape, tag, dtype=BF16):
                return wp.tile(shape, dtype, tag=tag, name=tag)

            def ps(shape, dtype=F32):
                return pp.tile(shape, dtype, tag="mm", name="mm")

            b1 = wt([128, B1_COLS], "b1")
            nc.sync.dma_start(out=b1[:, 0:B1_D1], in_=b1_d[:, 0:B1_D1])
            nc.scalar.dma_start(out=b1[:, B1_D1:B1_D2],
                                in_=b1_d[:, B1_D1:B1_D2])
            b2 = wt([128, 3], "b2", F32)
            nc.gpsimd.dma_start(out=b2, in_=b2_d[:, :])
            nc.gpsimd.dma_start(out=b1[:, B1_D2:B1_COLS],
                                in_=b1_d[:, B1_D2:B1_COLS])

            GA = b1[:, B1_GA:B1_GA + TAUG]
            hTp = b1[:, B1_HT:B1_HT + N]
            bT1m = b2[:, 0:1]
            btail = b2[0:TTL, 1:2]
            GB = b1[:, B1_GB:B1_GB + TAUG]
            aTp = b1[:, B1_AT:B1_AT + N]
            wenc = b1[:, B1_WENC:B1_WENC + 16]
            benc = b2[0:16, 2:3]
            mT = [b1[:, B1_MT0:B1_MT0 + N], b1[:, B1_MT1:B1_MT1 + N]]
            ident = b1[:, B1_ID:B1_ID + 128]

            # ---------- T1aug^T = G''aug^T [h;a]^T + bias ----------
            # obs+ones block lives at partitions 32:49 so S pass-2's lhsT
            # (T1aug tail rows 32:49) and rhs share a base partition
            ctT_full = wt([64, N], "ctT")
            nc.vector.memset(ctT_full, 1.0)
            ctT = ctT_full[32:49, :]
            pT1m = ps([128, N])
            nc.tensor.matmul(pT1m, GA[:, 0:128], hTp, start=True, stop=False)
            nc.tensor.matmul(pT1m, GB[:, 0:128], aTp, start=False, stop=True)
            pT1t = ps([TTL, N])
            nc.tensor.matmul(pT1t, GA[:, 128:TAUG], hTp, start=True, stop=False)
            nc.tensor.matmul(pT1t, GB[:, 128:TAUG], aTp, start=False, stop=True)
            pObs = ps([16, N])
            nc.tensor.matmul(pObs, wenc, hTp, start=True, stop=True)
            T1m = wt([128, N], "T1m")
            nc.vector.tensor_scalar(T1m, pT1m, bT1m, None, mybir.AluOpType.add)
            T1t = wt([TTL, N], "T1t")
            nc.vector.tensor_scalar(T1t, pT1t, btail, None, mybir.AluOpType.add)
            nc.scalar.activation(ctT_full[32:48, :], pObs,
                                 mybir.ActivationFunctionType.Identity,
                                 bias=benc, scale=1.0)

            # ---------- S = T1_jhalf C'^T -> E = exp(S/12) (2 halves) ----
            pS = ps([128, N])
            nc.tensor.matmul(pS, T1m[:, 0:128], aTp, start=True, stop=False)
            nc.tensor.matmul(pS, T1t[32:TTL, 0:128], ctT, start=False, stop=True)
            Et = wt([128, N], "Et")
            nc.scalar.activation(Et[:, 0:128], pS[:, 0:128],
                                 mybir.ActivationFunctionType.Exp, scale=SCALE)
            nc.scalar.activation(Et[:, 128:N], pS[:, 128:N],
                                 mybir.ActivationFunctionType.Exp, scale=SCALE)

            # ---------- u via PE transpose of T1aug tail rows 0:8 ----------
            u_t = []
            for kc in range(2):
                pu = ps([128, 8], BF16)
                nc.tensor.transpose(pu, T1t[0:8, kc * 128:(kc + 1) * 128],
                                    ident[0:8, 0:8])
                t = wt([128, 8], f"u{kc}")
                nc.vector.tensor_copy(out=t, in_=pu)
                u_t.append(t)

            # ---------- E^T via PE transpose (exp(S)^T == exp(S^T)) ----------
            ET = []
            for kc in range(2):
                pT = ps([128, 128], BF16)
                nc.tensor.transpose(pT, Et[:, kc * 128:(kc + 1) * 128], ident)
                t = wt([128, 128], f"ET{kc}")
                nc.vector.tensor_copy(out=t, in_=pT)
                ET.append(t)

            # ---------- D; R = mask^T * approx_recip(max(D, eps)) ----
            pD = ps([128, N])
            nc.tensor.matmul(pD, ET[0], mT[0], start=True, stop=False)
            nc.tensor.matmul(pD, ET[1], mT[1], start=False, stop=True)
            Rt = wt([128, N], "Rt", F32)
            nc.vector.tensor_scalar(Rt, pD, 1e-9, None, mybir.AluOpType.max)
            Rr = wt([128, N], "Rr", F32)
            nc.vector.reciprocal_approx_fast(out=Rr, in_=Rt)
            R = wt([128, N], "R")
            nc.vector.tensor_tensor(R, Rr, mT[0], mybir.AluOpType.mult)

            # ---------- W = mask^T * (E^T-partial over my j-half) ----------
            Wt = []
            for kc in range(2):
                pW = ps([128, N])
                nc.tensor.matmul(pW, Et[:, kc * 128:(kc + 1) * 128], R,
                                 start=True, stop=True)
                t = wt([128, N], f"W{kc}")
                nc.vector.tensor_tensor(t, pW, mT[kc], mybir.AluOpType.mult)
                Wt.append(t)

            # ---------- partial Q^T = u^T-contract with W : [8,256] ----------
            pQ = ps([8, N])
            nc.tensor.matmul(pQ, u_t[0], Wt[0], start=True, stop=False)
            nc.tensor.matmul(pQ, u_t[1], Wt[1], start=False, stop=True)
            Qsb = wt([8, N], "Qsb", F32)
            nc.vector.tensor_copy(out=Qsb, in_=pQ)
            nc.sync.dma_start(out=out_d[:, :], in_=Qsb[0:ACT, :])

    nc.compile()
    return nc


_NC_CACHE = {}


def _make_in_maps(inputs):
    f32 = np.float32
    g = lambda k: np.asarray(inputs[k], dtype=np.float64)

    hidden = np.asarray(inputs["hidden_state_n"], dtype=f32)
    action = np.asarray(inputs["action_n"], dtype=f32)
    state = np.asarray(inputs["state_n"]).astype(np.int64)

    # host-side weight folding (float64, cast at the end)
    Wq_eff = g("Wq") @ g("Wiq")
    bq_eff = g("bq") @ g("Wiq") + g("biq")
    Wk_eff = g("Wk") @ g("Wik")
    bk_eff = g("bk") @ g("Wik") + g("bik")
    Wv_eff = g("Wv") @ g("Wiv")
    bv_eff = g("bv") @ g("Wiv") + g("biv")
    Wo_eff = g("Wo_proj") @ g("W_O")          # [576,144]
    bo_eff = g("bo_proj") @ g("W_O")          # [144]
    W_adv = g("W_adv")
    W_Q = (g("W_val") @ np.ones((1, ACT)) + W_adv
           - (W_adv @ np.ones((ACT, ACT))) / ACT)              # [144,5]
    b_Q = g("b_val")[0] + g("b_adv") - g("b_adv").mean()       # [5]
    W_out = Wo_eff @ W_Q                                       # [576,5]
    c1 = (bo_eff @ W_Q).astype(f32)                            # [5]
    c2 = b_Q.astype(f32)                                       # [5]

    # mask from int state (host): mask[i,j] = j observed by i
    dx = np.abs(state[:, None, 0] - state[None, :, 0])
    dy = np.abs(state[:, None, 1] - state[None, :, 1])
    upper = np.arange(N)[None, :] > np.arange(N)[:, None]
    mask = ((dx <= 4) & (dy <= 2) & upper).astype(f32)         # [N,N]
    n_i = mask.sum(axis=1)                                     # [N]
    maskT = np.ascontiguousarray(mask.T)                       # [j,i]

    W_enc = g("W_enc")                                         # [128,16]
    b_enc = np.asarray(inputs["b_enc"], dtype=f32)             # [16]
    hT = np.ascontiguousarray(hidden.T)                        # [128,256]
    aT = np.ascontiguousarray(action.T)
    bf = lambda a: np.ascontiguousarray(np.asarray(a, np.float32)
                                        .astype(BF16NP))

    in_maps = []
    for c in range(NCORES):
        h, jm = c // 2, c % 2
        perm = np.roll(np.arange(N), -jm * 128)
        cols = slice(144 * h, 144 * h + 144)

        # A-mats in C'-feature row order [act(128), obs(16), ones(1)]
        def amat(W, b):
            Wh, bh = W[:, cols], b[cols]
            return np.vstack([Wh[16:144], Wh[0:16], bh[None, :]])  # [145,144]
        Aq, Ak, Av = amat(Wq_eff, bq_eff), amat(Wk_eff, bk_eff), \
            amat(Wv_eff, bv_eff)
        Gp = Aq @ Ak.T                                         # [145,145]
        Wv_aug = np.concatenate([Av @ W_out[cols, :],
                                 np.zeros((CF, 3))], axis=1)   # [145,8]
        # G'aug cols: T1 main(128) | u(8) | pad(24) | T1 tail(17)
        Gaug = np.concatenate([Gp[:, 0:128], Wv_aug,
                               np.zeros((CF, 24)), Gp[:, 128:145]], axis=1)
        GppA = W_enc @ Gaug[128:144, :]                        # hid rows [128,177]
        GppB = Gaug[0:128, :]                                  # act rows [128,177]
        bT1a = Gaug[144, :]                                    # [177]
        btail = np.zeros((128, 1))
        btail[0:TTL, 0] = bT1a[128:TAUG]
        mTp = maskT[perm, :]
        benc_col = np.zeros((128, 1))
        benc_col[0:16, 0] = b_enc

        b1 = np.concatenate([
            bf(GppA), bf(hT[:, perm]),
            bf(bT1a[0:128].reshape(128, 1)), bf(btail),
            bf(GppB), bf(aT[:, perm]), bf(W_enc), bf(benc_col),
            bf(mTp[0:128]), bf(mTp[128:256]),
            bf(np.eye(128))], axis=1)
        b2c = np.concatenate([bT1a[0:128].reshape(128, 1), btail,
                              benc_col], axis=1).astype(f32)
        in_maps.append({"blob1": np.ascontiguousarray(b1, dtype=BF16NP),
                        "blob2": np.ascontiguousarray(b2c, dtype=f32)})
    return in_maps, n_i, c1, c2


def kernel(**inputs):
    if "nc" not in _NC_CACHE:
        _NC_CACHE["nc"] = _build()
    nc = _NC_CACHE["nc"]
    in_maps, n_i, c1, c2 = _make_in_maps(inputs)
    res = bass_utils.run_bass_kernel_spmd(nc, in_maps, core_ids=list(range(NCORES)))
    QT = np.zeros((ACT, N), np.float32)
    for c in range(NCORES):
        QT += res.results[c]["out"]
    Q = QT.T + n_i[:, None] * c1[None, :] + c2[None, :]
    return Q.astype(np.float32)


# revision 16
# speedup vs baseline: 1.1239x; 1.0007x over previous
"""AttentionCritic Trainium2 kernel — 8-core SPMD, head/query-half sharded,
bf16 compute with fp32 PSUM accumulation.

Math restructuring (exact up to fp assoc./bf16 rounding):
  mask[i,j] = (|x_i-x_j|<=4)&(|y_i-y_j|<=2)&(j>i)          (host, from int state)
  C' = [act(128), obs(16), 1]  (ones feature folds all biases)
  S_h = C' G' C'^T / 12,  G' = Aq Ak^T host-folded from the two-stage
        reference projections (C@Wq+bq)@Wiq+biq -> single eff mats + bias rows
  T1aug = C' [G' | Wv_aug]  computed X-form: T1aug^T = G''^T [h;a]^T + bias
        (G'' absorbs the obs encoder; bias row added in the PSUM->SBUF copy).
        Wv_aug = Av (Wo_eff Wduel)_head, so T1aug's tail also carries
        u^T = (C' Wv_aug)^T — the v/ctx/out-proj fold rides along for free.
  E_h = exp(S_h)  (softmax ratio is shift-invariant; |S/12|<~3 so bf16
        logits cost <~1% on exp)
  E^T via PE transpose;  u via PE transpose of T1aug tail rows
  D[j,i] = sum_k E[j,k] mask[i,k] + 1e-9 (eps preloaded in PSUM)
  R = mask^T * approx_recip(D);  W[k,i] = mask[i,k] * sum_j E[j,k] R[j,i]
  Q_p^T[a,i] = sum_k u[k,a] W[k,i]
  Q = sum_cores Q_p^T + n_i*c1 + c2  (host)

Sharding: core c handles (head h=c//2, query-half jm=c%2). Everything after
exp is linear in j and h, so each core emits a partial Q^T [5,256] over ALL
256 agents and the host sums the 8 partials. The j-half selection is uniform
across cores: the per-core input packing rotates the agent axis by 128*jm,
so slice [0:128] is always "my" j-half.

Per-core: 17 matmuls, ~390KB DMA, all matmuls bf16 (1 cyc/row — fp32r runs
4x slower as fp32_mode=HIGH on this part), accumulation in fp32 PSUM.
"""

import sys

for _p in ("/opt/trn_rl_repo",):
    if _p not in sys.path:
        sys.path.append(_p)

import contextlib

import numpy as np
import ml_dtypes

import concourse.bass as bass
import concourse.bacc as bacc
import concourse.mybir as mybir
from concourse.tile import TileContext
from concourse import bass_utils

N, HID, ACT, NH = 256, 128, 5, 4
D, E, HD = 144, 576, 144
NCORES = 8
F32 = mybir.dt.float32
BF16 = mybir.dt.bfloat16
BF16NP = ml_dtypes.bfloat16
SCALE = 1.0 / 12.0
CF = 145   # C' feature dim: act(128) + obs(16) + ones(1)
# T1aug^T tail tile layout: rows 0:8 = u^T (padded from 5), rows 8:32 zero
# pad (transpose/matmul partition starts must be 32-aligned), rows 32:49 =
# T1 tail features (obs 16 + ones 1)
TAUG = 177  # G'aug cols: main(128) + u(8) + pad(24) + T1tail(17)
TTL = 49    # tail tile partitions

# blob1 [128, B1_COLS] bf16 column layout (host packing must match)
# dma1 (sync):   GA(177) hTp(256) bT1m(1) btail(1)      -> 0:435
# dma2 (scalar): GB(177) aTp(256) wenc(16) benc(1)      -> 435:885
# dma3 (gpsimd): mT0(256) mT1(256) id128(128)           -> 885:1525
B1_GA, B1_HT, B1_BT1M, B1_BTL = 0, 177, 433, 434
B1_D1 = 435
B1_GB, B1_AT, B1_WENC, B1_BENC = 435, 612, 868, 884
B1_D2 = 885
B1_MT0, B1_MT1, B1_ID = 885, 1141, 1397
B1_COLS = 1525


def _build():
    nc = bacc.Bacc(target_bir_lowering=False)

    b1_d = nc.declare_dram_parameter("blob1", [128, B1_COLS], BF16, False)
    b2_d = nc.declare_dram_parameter("blob2", [128, 3], F32, False)
    out_d = nc.declare_dram_parameter("out", [ACT, N], BF16, True)

    with TileContext(nc) as tc:
        with contextlib.ExitStack() as ctx:
            wp = ctx.enter_context(tc.tile_pool(name="wp", bufs=1))
            pp = ctx.enter_context(tc.tile_pool(name="pp", bufs=8, space="PSUM"))

            def wt(shape, tag, dtype=BF16):
                return wp.tile(shape, dtype, tag=tag, name=tag)

            def ps(shape, dtype=F32):
                return pp.tile(shape, dtype, tag="mm", name="mm")

            b1 = wt([128, B1_COLS], "b1")
            nc.sync.dma_start(out=b1[:, 0:B1_D1], in_=b1_d[:, 0:B1_D1])
            nc.scalar.dma_start(out=b1[:, B1_D1:B1_D2],
                                in_=b1_d[:, B1_D1:B1_D2])
            b2 = wt([128, 3], "b2", F32)
            nc.gpsimd.dma_start(out=b2, in_=b2_d[:, :])
            nc.gpsimd.dma_start(out=b1[:, B1_D2:B1_COLS],
                                in_=b1_d[:, B1_D2:B1_COLS])

            GA = b1[:, B1_GA:B1_GA + TAUG]
            hTp = b1[:, B1_HT:B1_HT + N]
            bT1m = b2[:, 0:1]
            btail = b2[0:TTL, 1:2]
            GB = b1[:, B1_GB:B1_GB + TAUG]
            aTp = b1[:, B1_AT:B1_AT + N]
            wenc = b1[:, B1_WENC:B1_WENC + 16]
            benc = b2[0:16, 2:3]
            mT = [b1[:, B1_MT0:B1_MT0 + N], b1[:, B1_MT1:B1_MT1 + N]]
            ident = b1[:, B1_ID:B1_ID + 128]

            # ---------- T1aug^T = G''aug^T [h;a]^T + bias ----------
            # obs+ones block lives at partitions 32:49 so S pass-2's lhsT
            # (T1aug tail rows 32:49) and rhs share a base partition
            ctT_full = wt([64, N], "ctT")
            nc.vector.memset(ctT_full, 1.0)
            ctT = ctT_full[32:49, :]
            pT1m = ps([128, N])
            nc.tensor.matmul(pT1m, GA[:, 0:128], hTp, start=True, stop=False)
            nc.tensor.matmul(pT1m, GB[:, 0:128], aTp, start=False, stop=True)
            pT1t = ps([TTL, N])
            nc.tensor.matmul(pT1t, GA[:, 128:TAUG], hTp, start=True, stop=False)
            nc.tensor.matmul(pT1t, GB[:, 128:TAUG], aTp, start=False, stop=True)
            pObs = ps([16, N])
            nc.tensor.matmul(pObs, wenc, hTp, start=True, stop=True)
            T1m = wt([128, N], "T1m")
            nc.vector.tensor_scalar(T1m, pT1m, bT1m, None, mybir.AluOpType.add)
            T1t = wt([TTL, N], "T1t")
            nc.vector.tensor_scalar(T1t, pT1t, btail, None, mybir.AluOpType.add)
            nc.scalar.activation(ctT_full[32:48, :], pObs,
                                 mybir.ActivationFunctionType.Identity,
                                 bias=benc, scale=1.0)

            # ---------- S = T1_jhalf C'^T -> E = exp(S/12) (2 halves) ----
            pS = ps([128, N])
            nc.tensor.matmul(pS, T1m[:, 0:128], aTp, start=True, stop=False)
            nc.tensor.matmul(pS, T1t[32:TTL, 0:128], ctT, start=False, stop=True)
            Et = wt([128, N], "Et")
            nc.scalar.activation(Et[:, 0:128], pS[:, 0:128],
                                 mybir.ActivationFunctionType.Exp, scale=SCALE)
            nc.scalar.activation(Et[:, 128:N], pS[:, 128:N],
                                 mybir.ActivationFunctionType.Exp, scale=SCALE)

            # ---------- u via PE transpose of T1aug tail rows 0:8 ----------
            u_t = []
            for kc in range(2):
                pu = ps([128, 8], BF16)
                nc.tensor.transpose(pu, T1t[0:8, kc * 128:(kc + 1) * 128],
                                    ident[0:8, 0:8])
                t = wt([128, 8], f"u{kc}")
                nc.vector.tensor_copy(out=t, in_=pu)
                u_t.append(t)

            # ---------- E^T via PE transpose (exp(S)^T == exp(S^T)) ----------
            ET = []
            for kc in range(2):
                pT = ps([128, 128], BF16)
                nc.tensor.transpose(pT, Et[:, kc * 128:(kc + 1) * 128], ident)
                t = wt([128, 128], f"ET{kc}")
                nc.vector.tensor_copy(out=t, in_=pT)
                ET.append(t)

            # ---------- D; R = mask^T * approx_recip(max(D, eps)) ----
            pD = ps([128, N])
            nc.tensor.matmul(pD, ET[0], mT[0], start=True, stop=False)
            nc.tensor.matmul(pD, ET[1], mT[1], start=False, stop=True)
            Rt = wt([128, N], "Rt", F32)
            nc.vector.tensor_scalar(Rt, pD, 1e-9, None, mybir.AluOpType.max)
            Rr = wt([128, N], "Rr", F32)
            nc.vector.reciprocal_approx_fast(out=Rr, in_=Rt)
            R = wt([128, N], "R")
            nc.vector.tensor_tensor(R, Rr, mT[0], mybir.AluOpType.mult)

            # ---------- W = mask^T * (E^T-partial over my j-half) ----------
            Wt = []
            for kc in range(2):
                pW = ps([128, N])
                nc.tensor.matmul(pW, Et[:, kc * 128:(kc + 1) * 128], R,
                                 start=True, stop=True)
                t = wt([128, N], f"W{kc}")
                nc.vector.tensor_tensor(t, pW, mT[kc], mybir.AluOpType.mult)
                Wt.append(t)

            # ---------- partial Q^T = u^T-contract with W : [8,256] ----------
            pQ = ps([8, N])
            nc.tensor.matmul(pQ, u_t[0], Wt[0], start=True, stop=False)
            nc.tensor.matmul(pQ, u_t[1], Wt[1], start=False, stop=True)
            Qsb = wt([8, N], "Qsb")
            nc.vector.tensor_copy(out=Qsb, in_=pQ)
            nc.sync.dma_start(out=out_d[:, :], in_=Qsb[0:ACT, :])

    nc.compile()
    return nc


_NC_CACHE = {}


def _make_in_maps(inputs):
    f32 = np.float32
    g = lambda k: np.asarray(inputs[k], dtype=np.float64)

    hidden = np.asarray(inputs["hidden_state_n"], dtype=f32)
    action = np.asarray(inputs["action_n"], dtype=f32)
    state = np.asarray(inputs["state_n"]).astype(np.int64)

    # host-side weight folding (float64, cast at the end)
    Wq_eff = g("Wq") @ g("Wiq")
    bq_eff = g("bq") @ g("Wiq") + g("biq")
    Wk_eff = g("Wk") @ g("Wik")
    bk_eff = g("bk") @ g("Wik") + g("bik")
    Wv_eff = g("Wv") @ g("Wiv")
    bv_eff = g("bv") @ g("Wiv") + g("biv")
    Wo_eff = g("Wo_proj") @ g("W_O")          # [576,144]
    bo_eff = g("bo_proj") @ g("W_O")          # [144]
    W_adv = g("W_adv")
    W_Q = (g("W_val") @ np.ones((1, ACT)) + W_adv
           - (W_adv @ np.ones((ACT, ACT))) / ACT)              # [144,5]
    b_Q = g("b_val")[0] + g("b_adv") - g("b_adv").mean()       # [5]
    W_out = Wo_eff @ W_Q                                       # [576,5]
    c1 = (bo_eff @ W_Q).astype(f32)                            # [5]
    c2 = b_Q.astype(f32)                                       # [5]

    # mask from int state (host): mask[i,j] = j observed by i
    dx = np.abs(state[:, None, 0] - state[None, :, 0])
    dy = np.abs(state[:, None, 1] - state[None, :, 1])
    upper = np.arange(N)[None, :] > np.arange(N)[:, None]
    mask = ((dx <= 4) & (dy <= 2) & upper).astype(f32)         # [N,N]
    n_i = mask.sum(axis=1)                                     # [N]
    maskT = np.ascontiguousarray(mask.T)                       # [j,i]

    W_enc = g("W_enc")                                         # [128,16]
    b_enc = np.asarray(inputs["b_enc"], dtype=f32)             # [16]
    hT = np.ascontiguousarray(hidden.T)                        # [128,256]
    aT = np.ascontiguousarray(action.T)
    bf = lambda a: np.ascontiguousarray(np.asarray(a, np.float32)
                                        .astype(BF16NP))

    in_maps = []
    for c in range(NCORES):
        h, jm = c // 2, c % 2
        perm = np.roll(np.arange(N), -jm * 128)
        cols = slice(144 * h, 144 * h + 144)

        # A-mats in C'-feature row order [act(128), obs(16), ones(1)]
        def amat(W, b):
            Wh, bh = W[:, cols], b[cols]
            return np.vstack([Wh[16:144], Wh[0:16], bh[None, :]])  # [145,144]
        Aq, Ak, Av = amat(Wq_eff, bq_eff), amat(Wk_eff, bk_eff), \
            amat(Wv_eff, bv_eff)
        Gp = Aq @ Ak.T                                         # [145,145]
        Wv_aug = np.concatenate([Av @ W_out[cols, :],
                                 np.zeros((CF, 3))], axis=1)   # [145,8]
        # G'aug cols: T1 main(128) | u(8) | pad(24) | T1 tail(17)
        Gaug = np.concatenate([Gp[:, 0:128], Wv_aug,
                               np.zeros((CF, 24)), Gp[:, 128:145]], axis=1)
        GppA = W_enc @ Gaug[128:144, :]                        # hid rows [128,177]
        GppB = Gaug[0:128, :]                                  # act rows [128,177]
        bT1a = Gaug[144, :]                                    # [177]
        btail = np.zeros((128, 1))
        btail[0:TTL, 0] = bT1a[128:TAUG]
        mTp = maskT[perm, :]
        benc_col = np.zeros((128, 1))
        benc_col[0:16, 0] = b_enc

        b1 = np.concatenate([
            bf(GppA), bf(hT[:, perm]),
            bf(bT1a[0:128].reshape(128, 1)), bf(btail),
            bf(GppB), bf(aT[:, perm]), bf(W_enc), bf(benc_col),
            bf(mTp[0:128]), bf(mTp[128:256]),
            bf(np.eye(128))], axis=1)
        b2c = np.concatenate([bT1a[0:128].reshape(128, 1), btail,
                              benc_col], axis=1).astype(f32)
        in_maps.append({"blob1": np.ascontiguousarray(b1, dtype=BF16NP),
                        "blob2": np.ascontiguousarray(b2c, dtype=f32)})
    return in_maps, n_i, c1, c2


def kernel(**inputs):
    if "nc" not in _NC_CACHE:
        _NC_CACHE["nc"] = _build()
    nc = _NC_CACHE["nc"]
    in_maps, n_i, c1, c2 = _make_in_maps(inputs)
    res = bass_utils.run_bass_kernel_spmd(nc, in_maps, core_ids=list(range(NCORES)))
    QT = np.zeros((ACT, N), np.float32)
    for c in range(NCORES):
        QT += np.asarray(res.results[c]["out"], np.float32)
    Q = QT.T + n_i[:, None] * c1[None, :] + c2[None, :]
    return Q.astype(np.float32)
